# revision 15
# baseline (speedup 1.0000x reference)
"""Causal self-attention with RoPE on 8 TRN2 NeuronCores.

Sharding: tensor-parallel over heads (H=8 -> 1 head per core).
v4: pipelined A(tb)=projections+rope / B(ib)=attention loop.
 - qk + swapped-weight qks matmuls (RoPE combine on DVE, as baseline).
 - v computed as v^T (wv stationary, N=512 matmuls), moved to (t, d)
   layout via one xbar DMA transpose per block + gpsimd strided copy.
 - exp(S^T/8) on [128,1024] pair tiles, 2/3 on ACT (true exp), 1/3 on
   DVE (Schraudolph bf16 bit-trick, round-to-nearest verified).
 - out_u + cs shipped to host; host divides and sums over heads.
"""
import sys

sys.path.insert(0, "/opt/trn_rl_repo")

import numpy as np
import ml_dtypes

import concourse.bass as bass
import concourse.mybir as mybir
import concourse.tile as tile
from concourse.bass_utils import run_bass_kernel_spmd

B, T, C, H = 1, 4096, 512, 8
HS = C // H  # 64
NCORES = 8
TB = 512           # t-block width for projections / i-block width for attention
NTB = T // TB      # 8
JC = 128           # j-chunk width
NJC = T // JC      # 32

SCH_A = float(16.0 * np.log2(np.e))   # Schraudolph slope (includes /8 scale)
SCH_B = 16250.375                     # RNE-optimal offset (bf16 bit space)
DVE_EXP_FRAC = 3                      # every 3rd pair -> DVE exp

_ctr = [0]


def _legalize_waits(nc):
    """This walrus build accepts at most one sem-wait command per hw
    instruction; move extra waits onto same-engine NoOps inserted before."""
    for f in nc.m.functions:
        for bb in f.blocks:
            insts = bb.instructions
            out = []
            for inst in insts:
                si = inst.sync_info
                if si is not None and len(si.on_wait) > 1:
                    waits = list(si.on_wait)
                    for w in waits[:-1]:
                        _ctr[0] += 1
                        nop = mybir.InstNoOp(name=f"I-waitsplit-{_ctr[0]}")
                        nop.engine = inst.engine
                        nop.sync_info = mybir.SyncInfo(on_wait=[w], on_update=[])
                        out.append(nop)
                    inst.sync_info = mybir.SyncInfo(
                        on_wait=[waits[-1]], on_update=list(si.on_update)
                    )
                out.append(inst)
            insts[:] = out
    return nc


def _build_nc():
    nc = bass.Bass()
    f32 = mybir.dt.float32
    bf16 = mybir.dt.bfloat16
    u16 = mybir.dt.uint16

    xt_in = nc.declare_dram_parameter("xt", [C, T], bf16, isOutput=False)
    wqk_in = nc.declare_dram_parameter("wqk", [C, 128], bf16, isOutput=False)
    wv_in = nc.declare_dram_parameter("wv", [C, HS], bf16, isOutput=False)
    wp_in = nc.declare_dram_parameter("wp", [HS, C], bf16, isOutput=False)
    cc_in = nc.declare_dram_parameter("cc", [128, T], bf16, isOutput=False)
    ss_in = nc.declare_dram_parameter("ss", [128, T], bf16, isOutput=False)
    out_u = nc.declare_dram_parameter("out_u", [T, C], bf16, isOutput=True)
    cs_out = nc.declare_dram_parameter("cs", [1, T], bf16, isOutput=True)

    Exp = mybir.ActivationFunctionType.Exp
    Mult = mybir.AluOpType.mult
    Add = mybir.AluOpType.add

    with tile.TileContext(nc) as tc:
        with (
            tc.tile_pool(name="big", bufs=1) as big,
            tc.tile_pool(name="ropet", bufs=2) as ropet,
            tc.tile_pool(name="vtsb", bufs=2) as vtsb,
            tc.tile_pool(name="vstg", bufs=2) as vstg,
            tc.tile_pool(name="ptp", bufs=5) as ptp,
            tc.tile_pool(name="ytsb", bufs=2) as ytsb,
            tc.tile_pool(name="outp", bufs=2) as outp,
            tc.tile_pool(name="mmp", bufs=2, space="PSUM") as mmp,
            tc.tile_pool(name="vpp", bufs=1, space="PSUM") as vpp,
            tc.tile_pool(name="stp", bufs=2, space="PSUM") as stp,
            tc.tile_pool(name="ytp", bufs=1, space="PSUM") as ytp,
        ):
            # ---- resident inputs; order DMAs so tb=0 deps come first ----
            xt_sb = big.tile([128, 4, T], bf16)
            _xt_r = xt_in.ap().rearrange("(n p) t -> p n t", p=128)
            wqk_sb = big.tile([128, 4, 128], bf16)
            wv_sb = big.tile([128, 4, HS], bf16)
            wp_sb = big.tile([HS, C], bf16)
            cc_sb = big.tile([128, T], bf16)
            ss_sb = big.tile([128, T], bf16)

            nc.sync.dma_start(out=xt_sb[:, :, 0:TB], in_=_xt_r[:, :, 0:TB])
            nc.sync.dma_start(out=wqk_sb, in_=wqk_in.ap().rearrange("(n p) m -> p n m", p=128))
            nc.sync.dma_start(out=cc_sb[:, 0:TB], in_=cc_in.ap()[:, 0:TB])
            nc.sync.dma_start(out=ss_sb[:, 0:TB], in_=ss_in.ap()[:, 0:TB])
            nc.sync.dma_start(out=wv_sb, in_=wv_in.ap().rearrange("(n p) m -> p n m", p=128))
            nc.sync.dma_start(out=wp_sb, in_=wp_in.ap())
            nc.sync.dma_start(out=xt_sb[:, :, TB:2 * TB], in_=_xt_r[:, :, TB:2 * TB])
            nc.sync.dma_start(out=cc_sb[:, TB:T], in_=cc_in.ap()[:, TB:T])
            nc.sync.dma_start(out=ss_sb[:, TB:T], in_=ss_in.ap()[:, TB:T])
            for _c in range(2, 8):
                _t0 = _c * TB
                nc.sync.dma_start(out=xt_sb[:, :, _t0:_t0 + TB],
                                  in_=_xt_r[:, :, _t0:_t0 + TB])

            qkr = big.tile([128, T], bf16)    # rows 0:64 q_rot^T, 64:128 k_rot^T
            krqr = big.tile([128, T], bf16)   # rows 0:64 k_rot^T, 64:128 q_rot^T
            v_ones = big.tile([128, NJC, HS + 1], bf16)
            nc.vector.memset(v_ones[:, :, HS], 1.0)

            exp_ctr = [0]
            out_pending = []  # deferred (ot, yt_sb, i0) output DMAs
            vstage_pending = []  # deferred (v_stage, tb) v_ones copies

            def flush_vstage(upto):
                while vstage_pending and vstage_pending[0][1] <= upto:
                    vs_, tb_ = vstage_pending.pop(0)
                    nc.gpsimd.tensor_copy(v_ones[:, 4 * tb_:4 * tb_ + 4, 0:HS], vs_)

            def flush_out():
                while out_pending:
                    ot_, yts_, i0_ = out_pending.pop(0)
                    nc.sync.dma_start(
                        out=out_u.ap()[i0_:i0_ + TB, :].rearrange("(q p) c -> p q c", p=128),
                        in_=ot_)
                    nc.sync.dma_start(out=cs_out.ap()[0:1, i0_:i0_ + TB],
                                      in_=yts_[HS:HS + 1, :])

            def phase_a(tb):
                tc0 = tb * TB
                qk_ps = mmp.tile([128, TB], f32, tag="mm")
                for cn in range(4):
                    nc.tensor.matmul(qk_ps, wqk_sb[:, cn, :], xt_sb[:, cn, tc0:tc0 + TB],
                                     start=(cn == 0), stop=(cn == 3))
                t2 = ropet.tile([128, TB], f32, tag="t2")
                nc.vector.tensor_mul(t2, qk_ps, cc_sb[:, tc0:tc0 + TB])
                w = ropet.tile([128, TB], f32, tag="w")
                nc.vector.tensor_mul(w, qk_ps, ss_sb[:, tc0:tc0 + TB])
                # RoPE pair-swap via stride-2-partition SBUF DMA
                ws = ropet.tile([128, TB], f32, tag="ws")
                w_r = w[:].rearrange("(a two) t -> a two t", two=2)
                ws_r = ws[:].rearrange("(a two) t -> a two t", two=2)
                nc.sync.dma_start(out=ws_r[:, 0, :], in_=w_r[:, 1, :])
                nc.sync.dma_start(out=ws_r[:, 1, :], in_=w_r[:, 0, :])
                # v^T = wv^T x  (wv stationary, N=512)
                v_ps = vpp.tile([HS, TB], f32, tag="vp")
                for cn in range(4):
                    nc.tensor.matmul(v_ps, wv_sb[:, cn, :], xt_sb[:, cn, tc0:tc0 + TB],
                                     start=(cn == 0), stop=(cn == 3))
                nc.vector.tensor_add(qkr[:, tc0:tc0 + TB], t2, ws)
                vt_sb = vtsb.tile([HS, TB], bf16, tag="vt")
                nc.vector.tensor_copy(vt_sb, v_ps)
                # swapped half-duplicate for the row-paired S^T matmuls
                nc.sync.dma_start(out=krqr[0:64, tc0:tc0 + TB], in_=qkr[64:128, tc0:tc0 + TB])
                nc.sync.dma_start(out=krqr[64:128, tc0:tc0 + TB], in_=qkr[0:64, tc0:tc0 + TB])
                # one xbar transpose (contiguous dst, row=q*128+p mapping);
                # the strided copy into the 65-wide v_ones layout is deferred
                # one iteration so its wait never blocks the Pool queue
                v_stage = vstg.tile([128, 4, HS], bf16, tag="vs")
                nc.sync.dma_start_transpose(v_stage[:], vt_sb[:])
                vstage_pending.append((v_stage, tb))

            def phase_b(ib):
                i0 = ib * TB
                nj = 4 * ib + 4
                flush_vstage(ib)
                yt_ps = ytp.tile([128, TB], f32, tag="yt")
                pend = []  # (pt, col_base, j) awaiting their PV matmul

                def flush_pv(n):
                    while len(pend) > n:
                        pt_, cb_, j_ = pend.pop(0)
                        v0_ = max(0, j_ * JC - i0)
                        nc.tensor.matmul(yt_ps[0:HS + 1, v0_:TB], v_ones[:, j_, :],
                                         pt_[:, cb_ + v0_:cb_ + TB],
                                         start=(j_ == 0), stop=(j_ == nj - 1),
                                         skip_group_check=True)

                for m in range(nj // 2):
                    j_e, j_o = 2 * m, 2 * m + 1
                    ve = max(0, j_e * JC - i0)
                    vo = max(0, j_o * JC - i0)
                    st = stp.tile([128, 2 * TB], f32, tag="st")
                    nc.tensor.matmul(st[:, ve:TB], krqr[0:64, j_e * JC:(j_e + 1) * JC],
                                     qkr[0:64, i0 + ve:i0 + TB], tile_position=(0, 0))
                    nc.tensor.matmul(st[:, TB + vo:2 * TB],
                                     qkr[64:128, j_o * JC:(j_o + 1) * JC],
                                     krqr[64:128, i0 + vo:i0 + TB], tile_position=(64, 0))
                    pt = ptp.tile([128, 2 * TB], bf16, tag="pt")
                    exp_ctr[0] += 1
                    if exp_ctr[0] % DVE_EXP_FRAC == 0:
                        nc.vector.tensor_scalar(pt[:, ve:2 * TB].bitcast(u16),
                                                st[:, ve:2 * TB], SCH_A, SCH_B,
                                                Mult, Add)
                    else:
                        nc.scalar.activation(pt[:, ve:2 * TB], st[:, ve:2 * TB],
                                             Exp, scale=0.125)
                    for cb, j, v0 in ((0, j_e, ve), (TB, j_o, vo)):
                        if j * JC + JC - 1 > i0:  # diagonal band elementwise mask
                            b0, b1 = v0, min(TB, v0 + JC)
                            nc.gpsimd.affine_select(
                                out=pt[:, cb + b0:cb + b1], in_=pt[:, cb + b0:cb + b1],
                                compare_op=mybir.AluOpType.is_ge,
                                fill=0.0, base=i0 + b0 - j * JC,
                                pattern=[[1, b1 - b0]], channel_multiplier=-1)
                        pend.append((pt, cb, j))
                    flush_pv(4)  # keep 2 pairs in flight so PE never waits on exp
                flush_pv(0)

                yt_sb = ytsb.tile([HS + 1, TB], bf16, tag="yts")
                nc.vector.tensor_copy(yt_sb, yt_ps[0:HS + 1, :])
                ot = outp.tile([128, 4, TB], bf16, tag="ot")
                for q in range(4):
                    op_ps = mmp.tile([128, TB], f32, tag="mm")
                    nc.tensor.matmul(op_ps, yt_sb[0:HS, q * 128:(q + 1) * 128], wp_sb)
                    if q < 2:
                        nc.scalar.copy(ot[:, q, :], op_ps)
                    else:
                        nc.vector.tensor_copy(ot[:, q, :], op_ps)
                flush_out()  # deferred by one ib: never blocks the Sync queue
                out_pending.append((ot, yt_sb, i0))

            phase_a(0)
            for ib in range(NTB):
                if ib + 1 < NTB:
                    phase_a(ib + 1)
                phase_b(ib)
            flush_out()

    _legalize_waits(nc)
    return nc


_cached = {}


def _get_nc():
    if "nc" not in _cached:
        _cached["nc"] = _build_nc()
    return _cached["nc"]


def _prep_inputs(x, rope, W_attn, W_proj):
    bf16 = ml_dtypes.bfloat16
    xt = np.ascontiguousarray(x[0].T).astype(bf16)          # (C, T)
    cos = np.asarray(rope[..., 0], dtype=np.float32)        # (T, HS//2)
    sin = np.asarray(rope[..., 1], dtype=np.float32)
    cc64 = np.repeat(cos.T, 2, axis=0)                      # (HS, T)
    ss64 = np.repeat(sin.T, 2, axis=0)
    ss64[1::2, :] *= -1.0        # sign for pair-swap form: odd rows -sin
    cc = np.ascontiguousarray(np.concatenate([cc64, cc64], axis=0)).astype(bf16)
    ss = np.ascontiguousarray(np.concatenate([ss64, ss64], axis=0)).astype(bf16)

    Wa = np.asarray(W_attn, dtype=np.float32)
    Wp = np.asarray(W_proj, dtype=np.float32)

    in_maps = []
    for h in range(NCORES):
        Wq = Wa[h * HS:(h + 1) * HS]                        # (HS, C)
        Wk = Wa[C + h * HS:C + (h + 1) * HS]
        Wv = Wa[2 * C + h * HS:2 * C + (h + 1) * HS]
        wqk = np.concatenate([Wq.T, Wk.T], axis=1).astype(bf16)        # (C, 128)
        wv = np.ascontiguousarray(Wv.T).astype(bf16)                   # (C, HS)
        wp = np.ascontiguousarray(Wp[:, h * HS:(h + 1) * HS].T).astype(bf16)  # (HS, C)
        in_maps.append({
            "xt": xt, "wqk": wqk, "wv": wv, "wp": wp, "cc": cc, "ss": ss,
        })
    return in_maps


def run_cores(x, rope, W_attn, W_proj, trace=False):
    """Returns BassKernelResults over the 8 cores."""
    nc = _get_nc()
    in_maps = _prep_inputs(x, rope, W_attn, W_proj)
    res = run_bass_kernel_spmd(nc, in_maps, list(range(NCORES)), trace=trace)
    return res


def kernel(x, rope, mask, W_attn, W_proj):
    res = run_cores(x, rope, W_attn, W_proj, trace=False)
    out = np.zeros((T, C), dtype=np.float32)
    for h in range(NCORES):
        r = res.results[h]
        cs = np.asarray(r["cs"], dtype=np.float32).reshape(T, 1)
        out += np.asarray(r["out_u"], dtype=np.float32) / cs
    return out.reshape(B, T, C).astype(np.float32)


# revision 16
# speedup vs baseline: 1.1437x; 1.1437x over previous
"""Causal self-attention with RoPE on 8 TRN2 NeuronCores.

Sharding: tensor-parallel over heads (H=8 -> 1 head per core).
v4: pipelined A(tb)=projections+rope / B(ib)=attention loop.
 - qk + swapped-weight qks matmuls (RoPE combine on DVE, as baseline).
 - v computed as v^T (wv stationary, N=512 matmuls), moved to (t, d)
   layout via one xbar DMA transpose per block + gpsimd strided copy.
 - exp(S^T/8) on [128,1024] pair tiles, 2/3 on ACT (true exp), 1/3 on
   DVE (Schraudolph bf16 bit-trick, round-to-nearest verified).
 - out_u + cs shipped to host; host divides and sums over heads.
"""
import sys

sys.path.insert(0, "/opt/trn_rl_repo")

import numpy as np
import ml_dtypes

import concourse.bass as bass
import concourse.mybir as mybir
import concourse.tile as tile
from concourse.bass_utils import run_bass_kernel_spmd

B, T, C, H = 1, 4096, 512, 8
HS = C // H  # 64
NCORES = 8
TB = 512           # t-block width for projections / i-block width for attention
NTB = T // TB      # 8
JC = 128           # j-chunk width
NJC = T // JC      # 32

SCH_A = float(16.0 * np.log2(np.e))   # Schraudolph slope (includes /8 scale)
SCH_B = 16250.375                     # RNE-optimal offset (bf16 bit space)
DVE_EXP_FRAC = 3                      # every 3rd pair -> DVE exp

_ctr = [0]


def _legalize_waits(nc):
    """This walrus build accepts at most one sem-wait command per hw
    instruction; move extra waits onto same-engine NoOps inserted before."""
    for f in nc.m.functions:
        for bb in f.blocks:
            insts = bb.instructions
            out = []
            for inst in insts:
                si = inst.sync_info
                if si is not None and len(si.on_wait) > 1:
                    waits = list(si.on_wait)
                    for w in waits[:-1]:
                        _ctr[0] += 1
                        nop = mybir.InstNoOp(name=f"I-waitsplit-{_ctr[0]}")
                        nop.engine = inst.engine
                        nop.sync_info = mybir.SyncInfo(on_wait=[w], on_update=[])
                        out.append(nop)
                    inst.sync_info = mybir.SyncInfo(
                        on_wait=[waits[-1]], on_update=list(si.on_update)
                    )
                out.append(inst)
            insts[:] = out
    return nc


def _build_nc():
    nc = bass.Bass()
    f32 = mybir.dt.float32
    bf16 = mybir.dt.bfloat16
    u16 = mybir.dt.uint16

    xt_in = nc.declare_dram_parameter("xt", [C, T], bf16, isOutput=False)
    wqk_in = nc.declare_dram_parameter("wqk", [C, 128], bf16, isOutput=False)
    wqks_in = nc.declare_dram_parameter("wqks", [C, 128], bf16, isOutput=False)
    wv_in = nc.declare_dram_parameter("wv", [C, HS], bf16, isOutput=False)
    wp_in = nc.declare_dram_parameter("wp", [HS, C], bf16, isOutput=False)
    cc_in = nc.declare_dram_parameter("cc", [128, T], bf16, isOutput=False)
    ss_in = nc.declare_dram_parameter("ss", [128, T], bf16, isOutput=False)
    out_u = nc.declare_dram_parameter("out_u", [T, C], bf16, isOutput=True)
    cs_out = nc.declare_dram_parameter("cs", [1, T], bf16, isOutput=True)

    Exp = mybir.ActivationFunctionType.Exp
    Mult = mybir.AluOpType.mult
    Add = mybir.AluOpType.add

    with tile.TileContext(nc) as tc:
        with (
            tc.tile_pool(name="big", bufs=1) as big,
            tc.tile_pool(name="ropet", bufs=2) as ropet,
            tc.tile_pool(name="ptp", bufs=5) as ptp,
            tc.tile_pool(name="ytsb", bufs=2) as ytsb,
            tc.tile_pool(name="outp", bufs=2) as outp,
            tc.tile_pool(name="mmp", bufs=2, space="PSUM") as mmp,
            tc.tile_pool(name="vpp", bufs=1, space="PSUM") as vpp,
            tc.tile_pool(name="stp", bufs=2, space="PSUM") as stp,
            tc.tile_pool(name="ytp", bufs=1, space="PSUM") as ytp,
        ):
            # ---- resident inputs; order DMAs so tb=0 deps come first ----
            xt_sb = big.tile([128, 4, T], bf16)
            _xt_r = xt_in.ap().rearrange("(n p) t -> p n t", p=128)
            wqk_sb = big.tile([128, 4, 128], bf16)
            wqks_sb = big.tile([128, 4, 128], bf16)
            wv_sb = big.tile([128, 4, HS], bf16)
            wp_sb = big.tile([HS, C], bf16)
            cc_sb = big.tile([128, T], bf16)
            ss_sb = big.tile([128, T], bf16)

            nc.sync.dma_start(out=xt_sb[:, :, 0:TB], in_=_xt_r[:, :, 0:TB])
            nc.sync.dma_start(out=wqk_sb, in_=wqk_in.ap().rearrange("(n p) m -> p n m", p=128))
            nc.sync.dma_start(out=wqks_sb, in_=wqks_in.ap().rearrange("(n p) m -> p n m", p=128))
            nc.sync.dma_start(out=cc_sb[:, 0:TB], in_=cc_in.ap()[:, 0:TB])
            nc.sync.dma_start(out=ss_sb[:, 0:TB], in_=ss_in.ap()[:, 0:TB])
            nc.sync.dma_start(out=wv_sb, in_=wv_in.ap().rearrange("(n p) m -> p n m", p=128))
            nc.sync.dma_start(out=wp_sb, in_=wp_in.ap())
            nc.sync.dma_start(out=xt_sb[:, :, TB:2 * TB], in_=_xt_r[:, :, TB:2 * TB])
            nc.sync.dma_start(out=cc_sb[:, TB:T], in_=cc_in.ap()[:, TB:T])
            nc.sync.dma_start(out=ss_sb[:, TB:T], in_=ss_in.ap()[:, TB:T])
            for _c in range(2, 8):
                _t0 = _c * TB
                nc.sync.dma_start(out=xt_sb[:, :, _t0:_t0 + TB],
                                  in_=_xt_r[:, :, _t0:_t0 + TB])

            qkr = big.tile([128, T], bf16)    # rows 0:64 q_rot^T, 64:128 k_rot^T
            krqr = big.tile([128, T], bf16)   # rows 0:64 k_rot^T, 64:128 q_rot^T
            v_ones = big.tile([128, NJC, HS + 1], bf16)
            nc.vector.memset(v_ones[:, :, HS], 1.0)

            exp_ctr = [0]
            out_pending = []  # deferred (ot, yt_sb, i0) output DMAs
            def flush_out():
                while out_pending:
                    ot_, yts_, i0_ = out_pending.pop(0)
                    nc.sync.dma_start(
                        out=out_u.ap()[i0_:i0_ + TB, :].rearrange("(q p) c -> p q c", p=128),
                        in_=ot_)
                    nc.sync.dma_start(out=cs_out.ap()[0:1, i0_:i0_ + TB],
                                      in_=yts_[HS:HS + 1, :])

            def phase_a(tb):
                tc0 = tb * TB
                qk_ps = mmp.tile([128, TB], f32, tag="mm")
                for cn in range(4):
                    nc.tensor.matmul(qk_ps, wqk_sb[:, cn, :], xt_sb[:, cn, tc0:tc0 + TB],
                                     start=(cn == 0), stop=(cn == 3))
                qks_ps = mmp.tile([128, TB], f32, tag="mm")
                for cn in range(4):
                    nc.tensor.matmul(qks_ps, wqks_sb[:, cn, :], xt_sb[:, cn, tc0:tc0 + TB],
                                     start=(cn == 0), stop=(cn == 3))
                t1 = ropet.tile([128, TB], f32, tag="t1")
                nc.vector.tensor_mul(t1, qks_ps, ss_sb[:, tc0:tc0 + TB])
                t2 = ropet.tile([128, TB], f32, tag="t2")
                nc.vector.tensor_mul(t2, qk_ps, cc_sb[:, tc0:tc0 + TB])
                nc.vector.tensor_add(qkr[:, tc0:tc0 + TB], t2, t1)
                # swapped half-duplicate for the row-paired S^T matmuls
                nc.sync.dma_start(out=krqr[0:64, tc0:tc0 + TB], in_=qkr[64:128, tc0:tc0 + TB])
                nc.sync.dma_start(out=krqr[64:128, tc0:tc0 + TB], in_=qkr[0:64, tc0:tc0 + TB])
                # v in (t, d) layout: 4 t-chunks into one packed PSUM tile
                v_ps = vpp.tile([128, 4, HS], f32, tag="vp")
                for t4 in range(4):
                    p0 = tc0 + t4 * 128
                    for cn in range(4):
                        nc.tensor.matmul(v_ps[:, t4, :], xt_sb[:, cn, p0:p0 + 128],
                                         wv_sb[:, cn, :],
                                         start=(cn == 0), stop=(cn == 3))
                nc.vector.tensor_copy(v_ones[:, 4 * tb:4 * tb + 4, 0:HS], v_ps)

            def phase_b(ib):
                i0 = ib * TB
                nj = 4 * ib + 4
                yt_ps = ytp.tile([128, TB], f32, tag="yt")
                pend = []  # (pt, col_base, j) awaiting their PV matmul

                def flush_pv(n):
                    while len(pend) > n:
                        pt_, cb_, j_ = pend.pop(0)
                        v0_ = max(0, j_ * JC - i0)
                        nc.tensor.matmul(yt_ps[0:HS + 1, v0_:TB], v_ones[:, j_, :],
                                         pt_[:, cb_ + v0_:cb_ + TB],
                                         start=(j_ == 0), stop=(j_ == nj - 1),
                                         skip_group_check=True)

                for m in range(nj // 2):
                    j_e, j_o = 2 * m, 2 * m + 1
                    ve = max(0, j_e * JC - i0)
                    vo = max(0, j_o * JC - i0)
                    st = stp.tile([128, 2 * TB], f32, tag="st")
                    nc.tensor.matmul(st[:, ve:TB], krqr[0:64, j_e * JC:(j_e + 1) * JC],
                                     qkr[0:64, i0 + ve:i0 + TB], tile_position=(0, 0))
                    nc.tensor.matmul(st[:, TB + vo:2 * TB],
                                     qkr[64:128, j_o * JC:(j_o + 1) * JC],
                                     krqr[64:128, i0 + vo:i0 + TB], tile_position=(64, 0))
                    pt = ptp.tile([128, 2 * TB], bf16, tag="pt")
                    exp_ctr[0] += 1
                    if 3 * m < nj // 2:  # first third of each ib's pairs -> DVE
                        nc.vector.tensor_scalar(pt[:, ve:2 * TB].bitcast(u16),
                                                st[:, ve:2 * TB], SCH_A, SCH_B,
                                                Mult, Add)
                    else:
                        nc.scalar.activation(pt[:, ve:2 * TB], st[:, ve:2 * TB],
                                             Exp, scale=0.125)
                    for cb, j, v0 in ((0, j_e, ve), (TB, j_o, vo)):
                        if j * JC + JC - 1 > i0:  # diagonal band elementwise mask
                            b0, b1 = v0, min(TB, v0 + JC)
                            nc.gpsimd.affine_select(
                                out=pt[:, cb + b0:cb + b1], in_=pt[:, cb + b0:cb + b1],
                                compare_op=mybir.AluOpType.is_ge,
                                fill=0.0, base=i0 + b0 - j * JC,
                                pattern=[[1, b1 - b0]], channel_multiplier=-1)
                        pend.append((pt, cb, j))
                    flush_pv(4)  # keep 2 pairs in flight so PE never waits on exp
                flush_pv(0)

                yt_sb = ytsb.tile([HS + 1, TB], bf16, tag="yts")
                nc.vector.tensor_copy(yt_sb, yt_ps[0:HS + 1, :])
                ot = outp.tile([128, 4, TB], bf16, tag="ot")
                for q in range(4):
                    op_ps = mmp.tile([128, TB], f32, tag="mm")
                    nc.tensor.matmul(op_ps, yt_sb[0:HS, q * 128:(q + 1) * 128], wp_sb)
                    if q < 2:
                        nc.scalar.copy(ot[:, q, :], op_ps)
                    else:
                        nc.vector.tensor_copy(ot[:, q, :], op_ps)
                flush_out()  # deferred by one ib: never blocks the Sync queue
                out_pending.append((ot, yt_sb, i0))

            phase_a(0)
            for ib in range(NTB):
                if ib + 1 < NTB:
                    phase_a(ib + 1)
                phase_b(ib)
            flush_out()

    _legalize_waits(nc)
    return nc


_cached = {}


def _get_nc():
    if "nc" not in _cached:
        _cached["nc"] = _build_nc()
    return _cached["nc"]


def _prep_inputs(x, rope, W_attn, W_proj):
    bf16 = ml_dtypes.bfloat16
    xt = np.ascontiguousarray(x[0].T).astype(bf16)          # (C, T)
    cos = np.asarray(rope[..., 0], dtype=np.float32)        # (T, HS//2)
    sin = np.asarray(rope[..., 1], dtype=np.float32)
    cc64 = np.repeat(cos.T, 2, axis=0)                      # (HS, T)
    ss64 = np.repeat(sin.T, 2, axis=0)
    ss64[0::2, :] *= -1.0        # sign folded: even rows -sin
    cc = np.ascontiguousarray(np.concatenate([cc64, cc64], axis=0)).astype(bf16)
    ss = np.ascontiguousarray(np.concatenate([ss64, ss64], axis=0)).astype(bf16)

    Wa = np.asarray(W_attn, dtype=np.float32)
    Wp = np.asarray(W_proj, dtype=np.float32)
    swap = np.arange(HS).reshape(-1, 2)[:, ::-1].reshape(-1)

    in_maps = []
    for h in range(NCORES):
        Wq = Wa[h * HS:(h + 1) * HS]                        # (HS, C)
        Wk = Wa[C + h * HS:C + (h + 1) * HS]
        Wv = Wa[2 * C + h * HS:2 * C + (h + 1) * HS]
        wqk = np.concatenate([Wq.T, Wk.T], axis=1).astype(bf16)        # (C, 128)
        wqks = np.concatenate([Wq[swap].T, Wk[swap].T], axis=1).astype(bf16)
        wv = np.ascontiguousarray(Wv.T).astype(bf16)                   # (C, HS)
        wp = np.ascontiguousarray(Wp[:, h * HS:(h + 1) * HS].T).astype(bf16)  # (HS, C)
        in_maps.append({
            "xt": xt, "wqk": wqk, "wqks": np.ascontiguousarray(wqks),
            "wv": wv, "wp": wp, "cc": cc, "ss": ss,
        })
    return in_maps


def run_cores(x, rope, W_attn, W_proj, trace=False):
    """Returns BassKernelResults over the 8 cores."""
    nc = _get_nc()
    in_maps = _prep_inputs(x, rope, W_attn, W_proj)
    res = run_bass_kernel_spmd(nc, in_maps, list(range(NCORES)), trace=trace)
    return res


def kernel(x, rope, mask, W_attn, W_proj):
    res = run_cores(x, rope, W_attn, W_proj, trace=False)
    out = np.zeros((T, C), dtype=np.float32)
    for h in range(NCORES):
        r = res.results[h]
        cs = np.asarray(r["cs"], dtype=np.float32).reshape(T, 1)
        out += np.asarray(r["out_u"], dtype=np.float32) / cs
    return out.reshape(B, T, C).astype(np.float32)


# revision 18
# speedup vs baseline: 1.2119x; 1.0596x over previous
"""Causal self-attention with RoPE on 8 TRN2 NeuronCores.

Sharding: tensor-parallel over heads (H=8 -> 1 head per core).
v4: pipelined A(tb)=projections+rope / B(ib)=attention loop.
 - qk + swapped-weight qks matmuls (RoPE combine on DVE, as baseline).
 - v computed as v^T (wv stationary, N=512 matmuls), moved to (t, d)
   layout via one xbar DMA transpose per block + gpsimd strided copy.
 - exp(S^T/8) on [128,1024] pair tiles, 2/3 on ACT (true exp), 1/3 on
   DVE (Schraudolph bf16 bit-trick, round-to-nearest verified).
 - out_u + cs shipped to host; host divides and sums over heads.
"""
import sys

sys.path.insert(0, "/opt/trn_rl_repo")

import numpy as np
import ml_dtypes

import concourse.bass as bass
import concourse.mybir as mybir
import concourse.tile as tile
from concourse.bass_utils import run_bass_kernel_spmd

B, T, C, H = 1, 4096, 512, 8
HS = C // H  # 64
NCORES = 8
TB = 512           # t-block width for projections / i-block width for attention
NTB = T // TB      # 8
JC = 128           # j-chunk width
NJC = T // JC      # 32

SCH_A = float(16.0 * np.log2(np.e))   # Schraudolph slope (includes /8 scale)
SCH_B = 16250.375                     # RNE-optimal offset (bf16 bit space)
DVE_EXP_FRAC = 3                      # every 3rd pair -> DVE exp

_ctr = [0]


def _legalize_waits(nc):
    """This walrus build accepts at most one sem-wait command per hw
    instruction; move extra waits onto same-engine NoOps inserted before."""
    for f in nc.m.functions:
        for bb in f.blocks:
            insts = bb.instructions
            out = []
            for inst in insts:
                si = inst.sync_info
                if si is not None and len(si.on_wait) > 1:
                    waits = list(si.on_wait)
                    for w in waits[:-1]:
                        _ctr[0] += 1
                        nop = mybir.InstNoOp(name=f"I-waitsplit-{_ctr[0]}")
                        nop.engine = inst.engine
                        nop.sync_info = mybir.SyncInfo(on_wait=[w], on_update=[])
                        out.append(nop)
                    inst.sync_info = mybir.SyncInfo(
                        on_wait=[waits[-1]], on_update=list(si.on_update)
                    )
                out.append(inst)
            insts[:] = out
    return nc


def _build_nc():
    nc = bass.Bass()
    f32 = mybir.dt.float32
    bf16 = mybir.dt.bfloat16
    u16 = mybir.dt.uint16

    xt_in = nc.declare_dram_parameter("xt", [C, T], bf16, isOutput=False)
    wqk_in = nc.declare_dram_parameter("wqk", [C, 128], bf16, isOutput=False)
    wqks_in = nc.declare_dram_parameter("wqks", [C, 128], bf16, isOutput=False)
    wv_in = nc.declare_dram_parameter("wv", [C, HS], bf16, isOutput=False)
    wp_in = nc.declare_dram_parameter("wp", [HS, C], bf16, isOutput=False)
    cc_in = nc.declare_dram_parameter("cc", [128, T], bf16, isOutput=False)
    ss_in = nc.declare_dram_parameter("ss", [128, T], bf16, isOutput=False)
    out_u = nc.declare_dram_parameter("out_u", [T, C], bf16, isOutput=True)
    cs_out = nc.declare_dram_parameter("cs", [1, T], bf16, isOutput=True)

    Exp = mybir.ActivationFunctionType.Exp
    Mult = mybir.AluOpType.mult
    Add = mybir.AluOpType.add

    with tile.TileContext(nc) as tc:
        with (
            tc.tile_pool(name="big", bufs=1) as big,
            tc.tile_pool(name="ropet", bufs=2) as ropet,
            tc.tile_pool(name="ptp", bufs=5) as ptp,
            tc.tile_pool(name="ytsb", bufs=2) as ytsb,
            tc.tile_pool(name="outp", bufs=2) as outp,
            tc.tile_pool(name="mmp", bufs=2, space="PSUM") as mmp,
            tc.tile_pool(name="vpp", bufs=1, space="PSUM") as vpp,
            tc.tile_pool(name="stp", bufs=2, space="PSUM") as stp,
            tc.tile_pool(name="ytp", bufs=1, space="PSUM") as ytp,
        ):
            # ---- resident inputs; order DMAs so tb=0 deps come first ----
            xt_sb = big.tile([128, 4, T], bf16)
            _xt_r = xt_in.ap().rearrange("(n p) t -> p n t", p=128)
            wqk_sb = big.tile([128, 4, 128], bf16)
            wqks_sb = big.tile([128, 4, 128], bf16)
            wv_sb = big.tile([128, 4, HS], bf16)
            wp_sb = big.tile([HS, C], bf16)
            cc_sb = big.tile([128, T], bf16)
            ss_sb = big.tile([128, T], bf16)

            nc.sync.dma_start(out=xt_sb[:, :, 0:TB], in_=_xt_r[:, :, 0:TB])
            nc.sync.dma_start(out=wqk_sb, in_=wqk_in.ap().rearrange("(n p) m -> p n m", p=128))
            nc.sync.dma_start(out=wqks_sb, in_=wqks_in.ap().rearrange("(n p) m -> p n m", p=128))
            nc.sync.dma_start(out=cc_sb[:, 0:TB], in_=cc_in.ap()[:, 0:TB])
            nc.sync.dma_start(out=ss_sb[:, 0:TB], in_=ss_in.ap()[:, 0:TB])
            nc.sync.dma_start(out=wv_sb, in_=wv_in.ap().rearrange("(n p) m -> p n m", p=128))
            nc.sync.dma_start(out=wp_sb, in_=wp_in.ap())
            nc.sync.dma_start(out=xt_sb[:, :, TB:2 * TB], in_=_xt_r[:, :, TB:2 * TB])
            nc.sync.dma_start(out=cc_sb[:, TB:T], in_=cc_in.ap()[:, TB:T])
            nc.sync.dma_start(out=ss_sb[:, TB:T], in_=ss_in.ap()[:, TB:T])
            for _c in range(2, 8):
                _t0 = _c * TB
                nc.sync.dma_start(out=xt_sb[:, :, _t0:_t0 + TB],
                                  in_=_xt_r[:, :, _t0:_t0 + TB])

            qkr = big.tile([128, T], bf16)    # rows 0:64 q_rot^T, 64:128 k_rot^T
            krqr = big.tile([128, T], bf16)   # rows 0:64 k_rot^T, 64:128 q_rot^T
            v_ones = big.tile([128, NJC, HS + 1], bf16)
            nc.vector.memset(v_ones[:, :, HS], 1.0)

            exp_ctr = [0]
            out_pending = []  # deferred (ot, yt_sb, i0) output DMAs
            def flush_out():
                while out_pending:
                    ot_, yts_, i0_ = out_pending.pop(0)
                    for q_ in range(4):
                        r0 = i0_ + q_ * 128
                        nc.sync.dma_start(out=out_u.ap()[r0:r0 + 128, :], in_=ot_[:, q_, :])
                    nc.sync.dma_start(out=cs_out.ap()[0:1, i0_:i0_ + TB],
                                      in_=yts_[HS:HS + 1, :])

            def phase_a(tb):
                tc0 = tb * TB
                qk_ps = mmp.tile([128, TB], f32, tag="mm")
                for cn in range(4):
                    nc.tensor.matmul(qk_ps, wqk_sb[:, cn, :], xt_sb[:, cn, tc0:tc0 + TB],
                                     start=(cn == 0), stop=(cn == 3))
                qks_ps = mmp.tile([128, TB], f32, tag="mm")
                for cn in range(4):
                    nc.tensor.matmul(qks_ps, wqks_sb[:, cn, :], xt_sb[:, cn, tc0:tc0 + TB],
                                     start=(cn == 0), stop=(cn == 3))
                t1 = ropet.tile([128, TB], f32, tag="t1")
                nc.vector.tensor_mul(t1, qks_ps, ss_sb[:, tc0:tc0 + TB])
                t2 = ropet.tile([128, TB], f32, tag="t2")
                nc.vector.tensor_mul(t2, qk_ps, cc_sb[:, tc0:tc0 + TB])
                nc.vector.tensor_add(qkr[:, tc0:tc0 + TB], t2, t1)
                # swapped half-duplicate for the row-paired S^T matmuls
                nc.sync.dma_start(out=krqr[0:64, tc0:tc0 + TB], in_=qkr[64:128, tc0:tc0 + TB])
                nc.sync.dma_start(out=krqr[64:128, tc0:tc0 + TB], in_=qkr[0:64, tc0:tc0 + TB])
                # v in (t, d) layout: 4 t-chunks into one packed PSUM tile
                v_ps = vpp.tile([128, 4, HS], f32, tag="vp")
                for t4 in range(4):
                    p0 = tc0 + t4 * 128
                    for cn in range(4):
                        nc.tensor.matmul(v_ps[:, t4, :], xt_sb[:, cn, p0:p0 + 128],
                                         wv_sb[:, cn, :],
                                         start=(cn == 0), stop=(cn == 3))
                nc.vector.tensor_copy(v_ones[:, 4 * tb:4 * tb + 4, 0:HS], v_ps)

            def phase_b(ib):
                i0 = ib * TB
                nj = 4 * ib + 4
                yt_ps = ytp.tile([128, TB], f32, tag="yt")
                pend = []  # (pt, col_base, j) awaiting their PV matmul

                def flush_pv(n):
                    while len(pend) > n:
                        pt_, cb_, j_ = pend.pop(0)
                        v0_ = max(0, j_ * JC - i0)
                        nc.tensor.matmul(yt_ps[0:HS + 1, v0_:TB], v_ones[:, j_, :],
                                         pt_[:, cb_ + v0_:cb_ + TB],
                                         start=(j_ == 0), stop=(j_ == nj - 1),
                                         skip_group_check=True)

                for m in range(nj // 2):
                    j_e, j_o = 2 * m, 2 * m + 1
                    ve = max(0, j_e * JC - i0)
                    vo = max(0, j_o * JC - i0)
                    st = stp.tile([128, 2 * TB], f32, tag="st")
                    nc.tensor.matmul(st[:, ve:TB], krqr[0:64, j_e * JC:(j_e + 1) * JC],
                                     qkr[0:64, i0 + ve:i0 + TB], tile_position=(0, 0))
                    nc.tensor.matmul(st[:, TB + vo:2 * TB],
                                     qkr[64:128, j_o * JC:(j_o + 1) * JC],
                                     krqr[64:128, i0 + vo:i0 + TB], tile_position=(64, 0))
                    pt = ptp.tile([128, 2 * TB], bf16, tag="pt")
                    exp_ctr[0] += 1
                    if m % 3 == 1:  # spread ~1/3 of each ib's pairs onto DVE
                        nc.vector.tensor_scalar(pt[:, ve:2 * TB].bitcast(u16),
                                                st[:, ve:2 * TB], SCH_A, SCH_B,
                                                Mult, Add)
                    else:
                        nc.scalar.activation(pt[:, ve:2 * TB], st[:, ve:2 * TB],
                                             Exp, scale=0.125)
                    for cb, j, v0 in ((0, j_e, ve), (TB, j_o, vo)):
                        if j * JC + JC - 1 > i0:  # diagonal band elementwise mask
                            b0, b1 = v0, min(TB, v0 + JC)
                            nc.gpsimd.affine_select(
                                out=pt[:, cb + b0:cb + b1], in_=pt[:, cb + b0:cb + b1],
                                compare_op=mybir.AluOpType.is_ge,
                                fill=0.0, base=i0 + b0 - j * JC,
                                pattern=[[1, b1 - b0]], channel_multiplier=-1)
                        pend.append((pt, cb, j))
                    flush_pv(4)  # keep 2 pairs in flight so PE never waits on exp
                flush_pv(0)

                yt_sb = ytsb.tile([HS + 1, TB], bf16, tag="yts")
                nc.vector.tensor_copy(yt_sb, yt_ps[0:HS + 1, :])
                ot = outp.tile([128, 4, TB], bf16, tag="ot")
                for q in range(4):
                    op_ps = mmp.tile([128, TB], f32, tag="mm")
                    nc.tensor.matmul(op_ps, yt_sb[0:HS, q * 128:(q + 1) * 128], wp_sb)
                    if q < 2:
                        nc.scalar.copy(ot[:, q, :], op_ps)
                    else:
                        nc.vector.tensor_copy(ot[:, q, :], op_ps)
                flush_out()  # deferred by one ib: never blocks the Sync queue
                out_pending.append((ot, yt_sb, i0))
                if ib == NTB - 1:
                    flush_out()  # last block: emit immediately, nothing follows
                if ib == NTB - 1:
                    flush_out()  # last block: emit immediately, nothing follows

            phase_a(0)
            for ib in range(NTB):
                if ib + 1 < NTB:
                    phase_a(ib + 1)
                phase_b(ib)
            flush_out()

    _legalize_waits(nc)
    return nc


_cached = {}


def _get_nc():
    if "nc" not in _cached:
        _cached["nc"] = _build_nc()
    return _cached["nc"]


def _prep_inputs(x, rope, W_attn, W_proj):
    bf16 = ml_dtypes.bfloat16
    xt = np.ascontiguousarray(x[0].T).astype(bf16)          # (C, T)
    cos = np.asarray(rope[..., 0], dtype=np.float32)        # (T, HS//2)
    sin = np.asarray(rope[..., 1], dtype=np.float32)
    cc64 = np.repeat(cos.T, 2, axis=0)                      # (HS, T)
    ss64 = np.repeat(sin.T, 2, axis=0)
    ss64[0::2, :] *= -1.0        # sign folded: even rows -sin
    cc = np.ascontiguousarray(np.concatenate([cc64, cc64], axis=0)).astype(bf16)
    ss = np.ascontiguousarray(np.concatenate([ss64, ss64], axis=0)).astype(bf16)

    Wa = np.asarray(W_attn, dtype=np.float32)
    Wp = np.asarray(W_proj, dtype=np.float32)
    swap = np.arange(HS).reshape(-1, 2)[:, ::-1].reshape(-1)

    in_maps = []
    for h in range(NCORES):
        Wq = Wa[h * HS:(h + 1) * HS]                        # (HS, C)
        Wk = Wa[C + h * HS:C + (h + 1) * HS]
        Wv = Wa[2 * C + h * HS:2 * C + (h + 1) * HS]
        wqk = np.concatenate([Wq.T, Wk.T], axis=1).astype(bf16)        # (C, 128)
        wqks = np.concatenate([Wq[swap].T, Wk[swap].T], axis=1).astype(bf16)
        wv = np.ascontiguousarray(Wv.T).astype(bf16)                   # (C, HS)
        wp = np.ascontiguousarray(Wp[:, h * HS:(h + 1) * HS].T).astype(bf16)  # (HS, C)
        in_maps.append({
            "xt": xt, "wqk": wqk, "wqks": np.ascontiguousarray(wqks),
            "wv": wv, "wp": wp, "cc": cc, "ss": ss,
        })
    return in_maps


def run_cores(x, rope, W_attn, W_proj, trace=False):
    """Returns BassKernelResults over the 8 cores."""
    nc = _get_nc()
    in_maps = _prep_inputs(x, rope, W_attn, W_proj)
    res = run_bass_kernel_spmd(nc, in_maps, list(range(NCORES)), trace=trace)
    return res


def kernel(x, rope, mask, W_attn, W_proj):
    res = run_cores(x, rope, W_attn, W_proj, trace=False)
    out = np.zeros((T, C), dtype=np.float32)
    for h in range(NCORES):
        r = res.results[h]
        cs = np.asarray(r["cs"], dtype=np.float32).reshape(T, 1)
        out += np.asarray(r["out_u"], dtype=np.float32) / cs
    return out.reshape(B, T, C).astype(np.float32)


# revision 20
# speedup vs baseline: 1.3649x; 1.1262x over previous
"""Causal self-attention with RoPE on 8 TRN2 NeuronCores.

Sharding: tensor-parallel over heads (H=8 -> 1 head per core).
v4: pipelined A(tb)=projections+rope / B(ib)=attention loop.
 - qk + swapped-weight qks matmuls (RoPE combine on DVE, as baseline).
 - v computed as v^T (wv stationary, N=512 matmuls), moved to (t, d)
   layout via one xbar DMA transpose per block + gpsimd strided copy.
 - exp(S^T/8) on [128,1024] pair tiles, 2/3 on ACT (true exp), 1/3 on
   DVE (Schraudolph bf16 bit-trick, round-to-nearest verified).
 - out_u + cs shipped to host; host divides and sums over heads.
"""
import sys

sys.path.insert(0, "/opt/trn_rl_repo")

import numpy as np
import ml_dtypes

import concourse.bass as bass
import concourse.mybir as mybir
import concourse.tile as tile
from concourse.bass_utils import run_bass_kernel_spmd

B, T, C, H = 1, 4096, 512, 8
HS = C // H  # 64
NCORES = 8
TB = 512           # t-block width for projections / i-block width for attention
NTB = T // TB      # 8
JC = 128           # j-chunk width
NJC = T // JC      # 32

SCH_A = float(16.0 * np.log2(np.e))   # Schraudolph slope (includes /8 scale)
SCH_B = 16250.375                     # RNE-optimal offset (bf16 bit space)
DVE_EXP_FRAC = 3                      # every 3rd pair -> DVE exp

_ctr = [0]


def _legalize_waits(nc):
    """This walrus build accepts at most one sem-wait command per hw
    instruction; move extra waits onto same-engine NoOps inserted before."""
    for f in nc.m.functions:
        for bb in f.blocks:
            insts = bb.instructions
            out = []
            for inst in insts:
                si = inst.sync_info
                if si is not None and len(si.on_wait) > 1:
                    waits = list(si.on_wait)
                    for w in waits[:-1]:
                        _ctr[0] += 1
                        nop = mybir.InstNoOp(name=f"I-waitsplit-{_ctr[0]}")
                        nop.engine = inst.engine
                        nop.sync_info = mybir.SyncInfo(on_wait=[w], on_update=[])
                        out.append(nop)
                    inst.sync_info = mybir.SyncInfo(
                        on_wait=[waits[-1]], on_update=list(si.on_update)
                    )
                out.append(inst)
            insts[:] = out
    return nc


def _build_nc():
    nc = bass.Bass()
    f32 = mybir.dt.float32
    bf16 = mybir.dt.bfloat16
    u16 = mybir.dt.uint16

    xt_in = nc.declare_dram_parameter("xt", [128, NTB, 4, TB], bf16, isOutput=False)
    wqk_in = nc.declare_dram_parameter("wqk", [C, 128], bf16, isOutput=False)
    wqks_in = nc.declare_dram_parameter("wqks", [C, 128], bf16, isOutput=False)
    wv_in = nc.declare_dram_parameter("wv", [C, HS], bf16, isOutput=False)
    wp_in = nc.declare_dram_parameter("wp", [HS, C], bf16, isOutput=False)
    cc_in = nc.declare_dram_parameter("cc", [128, T], bf16, isOutput=False)
    ss_in = nc.declare_dram_parameter("ss", [128, T], bf16, isOutput=False)
    out_u = nc.declare_dram_parameter("out_u", [128, NTB, 4, TB], bf16, isOutput=True)
    cs_out = nc.declare_dram_parameter("cs", [1, T], bf16, isOutput=True)

    Exp = mybir.ActivationFunctionType.Exp
    Mult = mybir.AluOpType.mult
    Add = mybir.AluOpType.add

    with tile.TileContext(nc) as tc:
        with (
            tc.tile_pool(name="big", bufs=1) as big,
            tc.tile_pool(name="ropet", bufs=2) as ropet,
            tc.tile_pool(name="ptp", bufs=5) as ptp,
            tc.tile_pool(name="ytsb", bufs=2) as ytsb,
            tc.tile_pool(name="outp", bufs=2) as outp,
            tc.tile_pool(name="mmp", bufs=2, space="PSUM") as mmp,
            tc.tile_pool(name="vpp", bufs=1, space="PSUM") as vpp,
            tc.tile_pool(name="stp", bufs=2, space="PSUM") as stp,
            tc.tile_pool(name="ytp", bufs=1, space="PSUM") as ytp,
        ):
            # ---- resident inputs; order DMAs so tb=0 deps come first ----
            xt_sb = big.tile([128, NTB, 4, TB], bf16)
            _xt_r = xt_in.ap()
            wqk_sb = big.tile([128, 4, 128], bf16)
            wqks_sb = big.tile([128, 4, 128], bf16)
            wv_sb = big.tile([128, 4, HS], bf16)
            wp_sb = big.tile([HS, C], bf16)
            cc_sb = big.tile([128, T], bf16)
            ss_sb = big.tile([128, T], bf16)

            nc.sync.dma_start(out=wqk_sb, in_=wqk_in.ap().rearrange("(n p) m -> p n m", p=128))
            nc.sync.dma_start(out=wqks_sb, in_=wqks_in.ap().rearrange("(n p) m -> p n m", p=128))
            nc.sync.dma_start(out=xt_sb[:, 0], in_=_xt_r[:, 0])
            nc.sync.dma_start(out=cc_sb[:, 0:TB], in_=cc_in.ap()[:, 0:TB])
            nc.sync.dma_start(out=ss_sb[:, 0:TB], in_=ss_in.ap()[:, 0:TB])
            nc.sync.dma_start(out=wv_sb, in_=wv_in.ap().rearrange("(n p) m -> p n m", p=128))
            nc.sync.dma_start(out=wp_sb, in_=wp_in.ap())
            nc.sync.dma_start(out=xt_sb[:, 1], in_=_xt_r[:, 1])
            nc.sync.dma_start(out=cc_sb[:, TB:T], in_=cc_in.ap()[:, TB:T])
            nc.sync.dma_start(out=ss_sb[:, TB:T], in_=ss_in.ap()[:, TB:T])
            for _c in range(2, 8):
                nc.sync.dma_start(out=xt_sb[:, _c], in_=_xt_r[:, _c])

            qkr = big.tile([128, T], bf16)    # rows 0:64 q_rot^T, 64:128 k_rot^T
            krqr = big.tile([128, T], bf16)   # rows 0:64 k_rot^T, 64:128 q_rot^T
            v_ones = big.tile([128, NJC, HS + 1], bf16)
            nc.vector.memset(v_ones[:, :, HS], 1.0)

            exp_ctr = [0]
            out_pending = []  # deferred (ot, yt_sb, i0) output DMAs
            cproj_pending = []  # deferred (yt_sb, i0) c_proj emissions

            def flush_cproj():
                while cproj_pending:
                    yts_, i0_ = cproj_pending.pop(0)
                    ot = outp.tile([128, 4, TB], bf16, tag="ot")
                    for q in range(4):
                        op_ps = mmp.tile([128, TB], f32, tag="mm")
                        nc.tensor.matmul(op_ps, yts_[0:HS, q * 128:(q + 1) * 128], wp_sb)
                        if q < 2:
                            nc.scalar.copy(ot[:, q, :], op_ps)
                        else:
                            nc.vector.tensor_copy(ot[:, q, :], op_ps)
                    out_pending.append((ot, yts_, i0_))
            def flush_out():
                while out_pending:
                    ot_, yts_, i0_ = out_pending.pop(0)
                    nc.sync.dma_start(out=out_u.ap()[:, i0_ // TB], in_=ot_)
                    nc.sync.dma_start(out=cs_out.ap()[0:1, i0_:i0_ + TB],
                                      in_=yts_[HS:HS + 1, :])

            def phase_a(tb):
                tc0 = tb * TB
                qk_ps = mmp.tile([128, TB], f32, tag="mm")
                for cn in range(4):
                    nc.tensor.matmul(qk_ps, wqk_sb[:, cn, :], xt_sb[:, tb, cn, :],
                                     start=(cn == 0), stop=(cn == 3))
                qks_ps = mmp.tile([128, TB], f32, tag="mm")
                for cn in range(4):
                    nc.tensor.matmul(qks_ps, wqks_sb[:, cn, :], xt_sb[:, tb, cn, :],
                                     start=(cn == 0), stop=(cn == 3))
                t1 = ropet.tile([128, TB], f32, tag="t1")
                nc.vector.tensor_mul(t1, qks_ps, ss_sb[:, tc0:tc0 + TB])
                t2 = ropet.tile([128, TB], f32, tag="t2")
                nc.vector.tensor_mul(t2, qk_ps, cc_sb[:, tc0:tc0 + TB])
                nc.vector.tensor_add(qkr[:, tc0:tc0 + TB], t2, t1)
                # swapped half-duplicate for the row-paired S^T matmuls
                nc.sync.dma_start(out=krqr[0:64, tc0:tc0 + TB], in_=qkr[64:128, tc0:tc0 + TB])
                nc.sync.dma_start(out=krqr[64:128, tc0:tc0 + TB], in_=qkr[0:64, tc0:tc0 + TB])
                # v in (t, d) layout: 4 t-chunks into one packed PSUM tile
                v_ps = vpp.tile([128, 4, HS], f32, tag="vp")
                for t4 in range(4):
                    for cn in range(4):
                        nc.tensor.matmul(v_ps[:, t4, :],
                                         xt_sb[:, tb, cn, t4 * 128:(t4 + 1) * 128],
                                         wv_sb[:, cn, :],
                                         start=(cn == 0), stop=(cn == 3))
                nc.vector.tensor_copy(v_ones[:, 4 * tb:4 * tb + 4, 0:HS], v_ps)

            def phase_b(ib):
                i0 = ib * TB
                nj = 4 * ib + 4
                flush_cproj()
                flush_out()
                yt_ps = ytp.tile([128, TB], f32, tag="yt")
                pend = []  # (pt, col_base, j) awaiting their PV matmul

                def flush_pv(n):
                    while len(pend) > n:
                        pt_, cb_, j_ = pend.pop(0)
                        v0_ = max(0, j_ * JC - i0)
                        nc.tensor.matmul(yt_ps[0:HS + 1, v0_:TB], v_ones[:, j_, :],
                                         pt_[:, cb_ + v0_:cb_ + TB],
                                         start=(j_ == 0), stop=(j_ == nj - 1),
                                         skip_group_check=True)

                for m in range(nj // 2):
                    j_e, j_o = 2 * m, 2 * m + 1
                    ve = max(0, j_e * JC - i0)
                    vo = max(0, j_o * JC - i0)
                    st = stp.tile([128, 2 * TB], f32, tag="st")
                    nc.tensor.matmul(st[:, ve:TB], krqr[0:64, j_e * JC:(j_e + 1) * JC],
                                     qkr[0:64, i0 + ve:i0 + TB], tile_position=(0, 0))
                    nc.tensor.matmul(st[:, TB + vo:2 * TB],
                                     qkr[64:128, j_o * JC:(j_o + 1) * JC],
                                     krqr[64:128, i0 + vo:i0 + TB], tile_position=(64, 0))
                    pt = ptp.tile([128, 2 * TB], bf16, tag="pt")
                    exp_ctr[0] += 1
                    if m % 3 == 1:  # spread ~1/3 of each ib's pairs onto DVE
                        nc.vector.tensor_scalar(pt[:, ve:2 * TB].bitcast(u16),
                                                st[:, ve:2 * TB], SCH_A, SCH_B,
                                                Mult, Add)
                    else:
                        nc.scalar.activation(pt[:, ve:2 * TB], st[:, ve:2 * TB],
                                             Exp, scale=0.125)
                    for cb, j, v0 in ((0, j_e, ve), (TB, j_o, vo)):
                        if j * JC + JC - 1 > i0:  # diagonal band elementwise mask
                            b0, b1 = v0, min(TB, v0 + JC)
                            nc.gpsimd.affine_select(
                                out=pt[:, cb + b0:cb + b1], in_=pt[:, cb + b0:cb + b1],
                                compare_op=mybir.AluOpType.is_ge,
                                fill=0.0, base=i0 + b0 - j * JC,
                                pattern=[[1, b1 - b0]], channel_multiplier=-1)
                        pend.append((pt, cb, j))
                    flush_pv(4)  # keep 2 pairs in flight so PE never waits on exp
                flush_pv(0)

                yt_sb = ytsb.tile([HS + 1, TB], bf16, tag="yts")
                nc.vector.tensor_copy(yt_sb, yt_ps[0:HS + 1, :])
                cproj_pending.append((yt_sb, i0))
                if ib == NTB - 1:
                    flush_cproj()
                    flush_out()
                if ib == NTB - 1:
                    flush_out()  # last block: emit immediately, nothing follows

            phase_a(0)
            for ib in range(NTB):
                if ib + 1 < NTB:
                    phase_a(ib + 1)
                phase_b(ib)
            flush_out()

    _legalize_waits(nc)
    return nc


_cached = {}


def _get_nc():
    if "nc" not in _cached:
        _cached["nc"] = _build_nc()
    return _cached["nc"]


def _prep_inputs(x, rope, W_attn, W_proj):
    bf16 = ml_dtypes.bfloat16
    # (C, T) -> [p, tb, n, t] so per-partition DMA runs are 4KB
    xt = np.ascontiguousarray(
        x[0].T.reshape(4, 128, NTB, TB).transpose(1, 2, 0, 3)).astype(bf16)
    cos = np.asarray(rope[..., 0], dtype=np.float32)        # (T, HS//2)
    sin = np.asarray(rope[..., 1], dtype=np.float32)
    cc64 = np.repeat(cos.T, 2, axis=0)                      # (HS, T)
    ss64 = np.repeat(sin.T, 2, axis=0)
    ss64[0::2, :] *= -1.0        # sign folded: even rows -sin
    cc = np.ascontiguousarray(np.concatenate([cc64, cc64], axis=0)).astype(bf16)
    ss = np.ascontiguousarray(np.concatenate([ss64, ss64], axis=0)).astype(bf16)

    Wa = np.asarray(W_attn, dtype=np.float32)
    Wp = np.asarray(W_proj, dtype=np.float32)
    swap = np.arange(HS).reshape(-1, 2)[:, ::-1].reshape(-1)

    in_maps = []
    for h in range(NCORES):
        Wq = Wa[h * HS:(h + 1) * HS]                        # (HS, C)
        Wk = Wa[C + h * HS:C + (h + 1) * HS]
        Wv = Wa[2 * C + h * HS:2 * C + (h + 1) * HS]
        wqk = np.concatenate([Wq.T, Wk.T], axis=1).astype(bf16)        # (C, 128)
        wqks = np.concatenate([Wq[swap].T, Wk[swap].T], axis=1).astype(bf16)
        wv = np.ascontiguousarray(Wv.T).astype(bf16)                   # (C, HS)
        wp = np.ascontiguousarray(Wp[:, h * HS:(h + 1) * HS].T).astype(bf16)  # (HS, C)
        in_maps.append({
            "xt": xt, "wqk": wqk, "wqks": np.ascontiguousarray(wqks),
            "wv": wv, "wp": wp, "cc": cc, "ss": ss,
        })
    return in_maps


def run_cores(x, rope, W_attn, W_proj, trace=False):
    """Returns BassKernelResults over the 8 cores."""
    nc = _get_nc()
    in_maps = _prep_inputs(x, rope, W_attn, W_proj)
    res = run_bass_kernel_spmd(nc, in_maps, list(range(NCORES)), trace=trace)
    return res


def kernel(x, rope, mask, W_attn, W_proj):
    res = run_cores(x, rope, W_attn, W_proj, trace=False)
    out = np.zeros((T, C), dtype=np.float32)
    for h in range(NCORES):
        r = res.results[h]
        cs = np.asarray(r["cs"], dtype=np.float32).reshape(T, 1)
        ou = np.asarray(r["out_u"], dtype=np.float32).transpose(1, 2, 0, 3).reshape(T, C)
        out += ou / cs
    return out.reshape(B, T, C).astype(np.float32)
